# revision 5
# baseline (speedup 1.0000x reference)
r"""DetCon (NT-Xent style) contrastive loss on 8 Trainium2 NeuronCores.

Reference computes, for v0/v1 = L2-normalized (over E) views scaled by
1/sqrt(T):   logits = [[S01, S00\diag], [S10, S11\diag]]  (2BN x 2BN-1)
             loss = mean_i( logsumexp(row_i) - label_logit_i )
with label_logit_i = S01[i,i] (== S10[i,i]).

Per-core plan (data-parallel over rows, host np.roll makes the program
core-independent):
  - load both views in natural [E, B*N] layout (2 x [128, 4096] f32 halves)
  - squares -> bf16 (gpsimd); column sumsq/10 broadcast to 128 partitions
    via all-0.1s [128,128] bf16 matmul (PE); 10/ss (DVE reciprocal);
    scale = sqrt (ACT) -> bf16; keys = raw*scale -> fp8e4 (gpsimd)
  - logits: fp8 DoubleRow matmuls (K=256 per instr) -> PSUM [128,2048]
  - row-sums of exp split across engines: ACT exp with fused accum_out on
    ~20 tiles; DVE on the rest via bf16 Schraudolph exp (tensor_scalar
    mult+add -> int16 bits, bitcast bf16, second 4x-mode tensor_scalar
    pass with accum_out); DVE scalar_tensor_tensor extracts diagonals
  - rowsum -= exp(diag_same_view)  (exact removal of the j==i term; all
    diag-bearing tiles are assigned to ACT so the exp paths match)
  - nll = ln(rowsum) - label; partition-reduce via ones-matmul -> scalar
Host sums the 8 per-core partial sums and divides by 2*B*N.
"""

import math
from contextlib import ExitStack

import numpy as np

import concourse.bacc as bacc
import concourse.bass as bass
import concourse.tile as tile
from concourse import mybir
from concourse.bass_utils import run_bass_kernel_spmd

B, E, N = 64, 256, 64
BN = B * N            # 4096 rows per view
NCORES = 8
CHUNK = BN // NCORES  # 512 rows (of each view) per core
P = 128
KH = E // P           # 2 contraction halves
G = 2048              # column group width (PSUM tile free dim)
NG = BN // G          # 2 column groups
TEMP = 0.1

# bf16 Schraudolph exp: bits(e^x) ~= int16(x * 128/ln2 + (16256 + C))
SCH_A = 184.6649652337873
SCH_C = -5.0          # bias-minimizing offset (calibrated in numpy)
SCH_B = 16256.0 + SCH_C

# exp-tile engine assignment per (tg, g) quad, indexed by hm parity.
# 'A' = ACT exact exp, 'V' = DVE Schraudolph. (tg=1, g=0) holds the
# same-view diagonal and must stay on ACT.
ASSIGN = {
    0: {(0, 0): 'V', (0, 1): 'A', (1, 0): 'A', (1, 1): 'A'},
    1: {(0, 0): 'V', (0, 1): 'V', (1, 0): 'A', (1, 1): 'A'},
}

F32 = mybir.dt.float32
BF16 = mybir.dt.bfloat16
FP8 = mybir.dt.float8e4
I16 = mybir.dt.int16


def _emit_pass(nc, pl, vin, out_dram, r, do_setup=True, do_main=True,
               nrm_prev=None):
    """Emit one full loss computation (rep r, for timing replication)."""
    if not do_setup:
        nrm8 = nrm_prev
    else:
        nrm8 = _emit_setup(nc, pl, vin, r)
    if not do_main:
        return nrm8

    _emit_main(nc, pl, out_dram, r, nrm8)
    return nrm8


def _emit_setup(nc, pl, vin, r):
    ident, ones01, ones_col = pl["consts"]
    # ---- load raw views in [E, B*N] layout (two 128-partition halves),
    # split per column-group across both HWDGE engines ----
    raw = [[None] * KH for _ in range(2)]
    GB = B // NG  # b-range per column group
    for v in range(2):
        for h in range(KH):
            t = pl["raw"].tile([P, BN], F32, tag=f"raw{v}{h}",
                               name=f"raw{v}{h}_{r}")
            for g in range(NG):
                src = vin[v][g * GB:(g + 1) * GB, h * P:(h + 1) * P, :] \
                    .rearrange("b e n -> e b n")
                dst = t[:, g * G:(g + 1) * G].rearrange(
                    "e (b n) -> e b n", b=GB)
                eng = nc.sync if (v + h) % 2 == 0 else nc.scalar
                eng.dma_start(out=dst, in_=src)
            raw[v][h] = t

    # ---- normalize: scale = sqrt(10/sumsq) broadcast on all partitions,
    # keys = raw * scale downcast to fp8 (k-interleaved for DoubleRow) ----
    nrm8 = [pl["nrm"].tile([P, KH, BN], FP8, tag=f"nrm{v}",
                           name=f"nrm{v}_{r}") for v in range(2)]
    for g in range(NG):
        gs = slice(g * G, (g + 1) * G)
        for v in range(2):
            sq = [pl["sq"].tile([P, G], BF16, tag=f"sq{h}",
                                name=f"sq{v}{g}{h}_{r}") for h in range(KH)]
            for h in range(KH):
                nc.gpsimd.tensor_mul(
                    sq[h][:], raw[v][h][:, gs], raw[v][h][:, gs])
            ssb = pl["psum"].tile([P, G], F32, tag="ps", name=f"ssb{v}{g}_{r}")
            for j in range(G // 512):
                js = slice(j * 512, (j + 1) * 512)
                for h in range(KH):
                    nc.tensor.matmul(
                        ssb[:, js], ones01[:], sq[h][:, js],
                        start=(h == 0), stop=(h == KH - 1))
            rb = pl["vec"].tile([P, G], F32, tag="rb", name=f"rb{v}{g}_{r}")
            nc.vector.reciprocal(rb[:], ssb[:])
            scl = pl["scl"].tile([P, G], BF16, tag="scl", name=f"scl{v}{g}_{r}")
            nc.scalar.activation(
                scl[:], rb[:], mybir.ActivationFunctionType.Sqrt)
            for h in range(KH):
                nc.gpsimd.tensor_mul(
                    nrm8[v][:, h, gs], raw[v][h][:, gs], scl[:])
    return nrm8


def _emit_main(nc, pl, out_dram, r, nrm8):
    ident, ones01, ones_col = pl["consts"]
    # per-pass collectors
    stats = pl["sml"].tile([P, 32], F32, tag="stats", name=f"stats{r}")
    diag01 = pl["sml"].tile([P, 8], F32, tag="diag01", name=f"diag01{r}")
    diag00 = pl["sml"].tile([P, 8], F32, tag="diag00", name=f"diag00{r}")
    # ---- main: row-block outer (weight reuse), then tg/g column tiles ----
    for half in range(2):       # 0: v0 rows, 1: v1 rows
        for m in range(4):      # 128-row blocks of this core's chunk
            hm = half * 4 + m
            ms = slice(m * P, (m + 1) * P)
            lhsT = nrm8[half][:, :, ms]
            for tg in range(2):  # 0: cross-view keys, 1: same-view
                keys = nrm8[1 - half] if tg == 0 else nrm8[half]
                for g in range(NG):
                    goff = g * G
                    pt = pl["psum"].tile([P, G], F32, tag="ps",
                                         name=f"pt{g}{hm}{tg}_{r}")
                    for j in range(G // 512):
                        js = slice(j * 512, (j + 1) * 512)
                        nc.tensor.matmul(
                            pt[:, js], lhsT,
                            keys[:, :, goff + j * 512:goff + (j + 1) * 512],
                            perf_mode=mybir.MatmulPerfMode.DoubleRow)
                    if g == 0:
                        # tg==0: label logit (cross-view diag); tg==1:
                        # same-view diag (removed from row-sum later)
                        dst = diag01 if tg == 0 else diag00
                        dsc = pl["dsc"].tile([P, P], BF16, tag="dsc",
                                             name=f"dsc{hm}{tg}_{r}")
                        nc.vector.scalar_tensor_tensor(
                            dsc[:], pt[:, ms], 1.0, ident[:],
                            op0=mybir.AluOpType.mult,
                            op1=mybir.AluOpType.mult,
                            accum_out=dst[:, hm:hm + 1])
                    sidx = hm * 4 + tg * 2 + g
                    if ASSIGN[hm % 2][(tg, g)] == 'A':
                        esc = pl["esc"].tile([P, G], BF16, tag="esc",
                                             name=f"esc{g}{hm}{tg}_{r}")
                        nc.scalar.activation(
                            esc[:], pt[:, :],
                            mybir.ActivationFunctionType.Exp,
                            accum_out=stats[:, sidx:sidx + 1])
                    else:
                        it = pl["i16"].tile([P, G], I16, tag="i16",
                                            name=f"it{g}{hm}{tg}_{r}")
                        nc.vector.tensor_scalar(
                            it[:], pt[:, :], SCH_A, SCH_B,
                            op0=mybir.AluOpType.mult,
                            op1=mybir.AluOpType.add)
                        bfv = it[:].bitcast(BF16)
                        nc.vector.tensor_scalar(
                            bfv, bfv, 1.0, 0.0,
                            op0=mybir.AluOpType.mult,
                            op1=mybir.AluOpType.add,
                            accum_out=stats[:, sidx:sidx + 1])

    # ---- epilogue: nll partial sum ----
    ediag = pl["sml"].tile([P, 8], F32, tag="ediag", name=f"ediag{r}")
    nc.scalar.activation(ediag[:], diag00[:], mybir.ActivationFunctionType.Exp)
    rows = pl["sml"].tile([P, 8], F32, tag="rows", name=f"rows{r}")
    nc.vector.tensor_reduce(
        rows[:], stats[:].rearrange("p (m t) -> p m t", t=4),
        axis=mybir.AxisListType.X, op=mybir.AluOpType.add)
    nc.vector.tensor_sub(rows[:], rows[:], ediag[:])
    lnr = pl["sml"].tile([P, 8], F32, tag="lnr", name=f"lnr{r}")
    lnsum = pl["sml"].tile([P, 1], F32, tag="lnsum", name=f"lnsum{r}")
    nc.scalar.activation(
        lnr[:], rows[:], mybir.ActivationFunctionType.Ln, accum_out=lnsum[:])
    dsum = pl["sml"].tile([P, 1], F32, tag="dsum", name=f"dsum{r}")
    nc.vector.tensor_reduce(
        dsum[:], diag01[:], axis=mybir.AxisListType.X, op=mybir.AluOpType.add)
    tot = pl["sml"].tile([P, 1], F32, tag="tot", name=f"tot{r}")
    nc.vector.tensor_sub(tot[:], lnsum[:], dsum[:])
    fp = pl["psum"].tile([P, G], F32, tag="ps", name=f"fp{r}")
    nc.tensor.matmul(fp[0:1, 0:1], tot[:], ones_col[:])
    res = pl["sml"].tile([1, 1], F32, tag="res", name=f"res{r}")
    nc.vector.tensor_copy(res[:], fp[0:1, 0:1])
    nc.sync.dma_start(out=out_dram[:], in_=res[:])


def _build_nc(reps: int = 1, mode: str = "full"):
    """mode: 'full' reps everything; 'main' reps only the logits+exp phase
    (one shared setup); 'setup' reps only load+normalize."""
    nc = bacc.Bacc()
    vin = [
        nc.dram_tensor("view0", [B, E, N], F32, kind="ExternalInput"),
        nc.dram_tensor("view1", [B, E, N], F32, kind="ExternalInput"),
    ]
    ident_in = nc.dram_tensor("ident", [P, P], F32, kind="ExternalInput")
    out_dram = nc.dram_tensor("out", [1, 1], F32, kind="ExternalOutput")

    with ExitStack() as ctx:
        tc = ctx.enter_context(tile.TileContext(nc))
        pl = {
            name: ctx.enter_context(tc.tile_pool(name=name, bufs=bufs))
            for name, bufs in (("raw", 1), ("sq", 2), ("nrm", 2), ("vec", 2),
                               ("scl", 2), ("esc", 2), ("i16", 2), ("dsc", 2),
                               ("sml", 1))
        }
        pl["psum"] = ctx.enter_context(
            tc.tile_pool(name="psum", bufs=2, space="PSUM"))

        ident = pl["sml"].tile([P, P], F32, tag="ident", name="ident")
        nc.sync.dma_start(out=ident[:], in_=ident_in[:])
        ones01 = pl["sml"].tile([P, P], BF16, tag="ones01", name="ones01")
        nc.vector.memset(ones01[:], 0.1)
        ones_col = pl["sml"].tile([P, 1], F32, tag="ones_col", name="ones_col")
        nc.vector.memset(ones_col[:], 1.0)
        pl["consts"] = (ident, ones01, ones_col)

        nrm = None
        for r in range(reps):
            nrm = _emit_pass(
                nc, pl, vin, out_dram, r,
                do_setup=(mode != "main" or r == 0),
                do_main=(mode != "setup"),
                nrm_prev=nrm)

    nc.compile()
    return nc


_NC_CACHE = None


def _run_spmd(view0: np.ndarray, view1: np.ndarray, nc=None, **spmd_kwargs):
    global _NC_CACHE
    if nc is None:
        if _NC_CACHE is None:
            _NC_CACHE = _build_nc()
        nc = _NC_CACHE

    ident = np.eye(P, dtype=np.float32)
    in_maps = []
    for c in range(NCORES):
        in_maps.append({
            "view0": np.ascontiguousarray(np.roll(view0, -c * (B // NCORES), axis=0)),
            "view1": np.ascontiguousarray(np.roll(view1, -c * (B // NCORES), axis=0)),
            "ident": ident,
        })
    res = run_bass_kernel_spmd(nc, in_maps, core_ids=list(range(NCORES)),
                               **spmd_kwargs)
    total = sum(float(r["out"][0, 0]) for r in res.results)
    return np.float32(total / (2 * BN)), res


def kernel(view0: np.ndarray, view1: np.ndarray) -> np.ndarray:
    loss, _ = _run_spmd(view0, view1)
    return loss


# revision 10
# speedup vs baseline: 2.3000x; 2.3000x over previous
r"""DetCon (NT-Xent style) contrastive loss on 8 Trainium2 NeuronCores.

Reference computes, for v0/v1 = L2-normalized (over E) views scaled by
1/sqrt(T):   logits = [[S01, S00\diag], [S10, S11\diag]]  (2BN x 2BN-1)
             loss = mean_i( logsumexp(row_i) - label_logit_i )
with label_logit_i = S01[i,i] (== S10[i,i]).

Per-core plan (data-parallel over rows, host np.roll makes the program
core-independent):
  - load both views in natural [E, B*N] layout (2 x [128, 4096] f32 halves)
  - normalize, per column group g (column-group outer so group-1 normalize
    overlaps group-0 logits): squares -> bf16 (gpsimd); sumsq/10 broadcast
    to all partitions via all-0.1s [128,128] bf16 matmul (PE) into a
    dedicated 1-bank PSUM ring; 10/ss (DVE reciprocal); scale = sqrt (ACT)
    -> bf16; keys = raw*scale -> fp8e4 (gpsimd), k-interleaved for DoubleRow
  - logits: fp8 DoubleRow matmuls (K=256/instr) -> [128,1024] PSUM tiles in
    a 3-deep ring so PE, ACT and DVE all overlap
  - row-sums of exp split across engines: ACT exp with fused accum_out on
    ~60% of tiles; DVE on the rest via bf16 Schraudolph exp (tensor_scalar
    mult+add -> int16 bits, bitcast bf16, 4x-mode tensor_scalar accum pass);
    DVE scalar_tensor_tensor extracts diagonals from PSUM
  - rowsum -= exp(diag_same_view)  (exact removal of the j==i term; the
    diag-bearing half-tiles are pinned to ACT so the exp paths match)
  - nll = ln(rowsum) - label; partition-reduce via ones-matmul -> scalar
Host sums the 8 per-core partial sums and divides by 2*B*N.
"""

import math
from contextlib import ExitStack

import numpy as np

import concourse.bacc as bacc
import concourse.bass as bass
import concourse.tile as tile
from concourse import mybir
from concourse.bass_utils import run_bass_kernel_spmd

B, E, N = 64, 256, 64
BN = B * N            # 4096 rows per view
NCORES = 8
CHUNK = BN // NCORES  # 512 rows (of each view) per core
P = 128
KH = E // P           # 2 contraction halves
G = 2048              # column group width
NG = BN // G          # 2 column groups
HT = 1024             # main PSUM half-tile width (2 banks)
TEMP = 0.1

# bf16 Schraudolph exp: bits(e^x) ~= int16(x * 128/ln2 + (16256 + C))
SCH_A = 184.6649652337873
SCH_C = -5.0
SCH_B = 16256.0 + SCH_C

F32 = mybir.dt.float32
BF16 = mybir.dt.bfloat16
FP8 = mybir.dt.float8e4
I16 = mybir.dt.int16


def _build_assign():
    """Engine per half-tile key (g, half, m, tg, c): 'A' (ACT exp) or
    'V' (DVE Schraudolph). (g0, tg1, c0) holds the same-view diagonal and
    must be ACT (matches the ACT ediag exp). Target ~60% ACT, interleaved."""
    asg = {}
    quota = {'A': 0, 'V': 0}
    for g in range(NG):
        for half in range(2):
            for m in range(4):
                for tg in range(2):
                    for c in range(2):
                        key = (g, half, m, tg, c)
                        if g == 0 and tg == 1 and c == 0:
                            e = 'A'
                        else:
                            # keep global ratio 5A:3V while alternating
                            e = 'A' if 3 * quota['A'] <= 5 * quota['V'] else 'V'
                        asg[key] = e
                        quota[e] += 1
    return asg


ASSIGN = _build_assign()


def _emit_load_g(nc, pl, vin, r, g, raw):
    GB = B // NG
    for v in range(2):
        for h in range(KH):
            if raw[v][h] is None:
                raw[v][h] = pl["raw"].tile([P, BN], F32, tag=f"raw{v}{h}",
                                           name=f"raw{v}{h}_{r}")
            t = raw[v][h]
            src = vin[v][g * GB:(g + 1) * GB, h * P:(h + 1) * P, :] \
                .rearrange("b e n -> e b n")
            dst = t[:, g * G:(g + 1) * G].rearrange("e (b n) -> e b n", b=GB)
            eng = nc.sync if (v + h) % 2 == 0 else nc.scalar
            eng.dma_start(out=dst, in_=src)


def _emit_norm_g(nc, pl, r, g, raw, nrm8):
    ident, ones01, ones_col = pl["consts"]
    gs = slice(g * G, (g + 1) * G)
    for v in range(2):
        if nrm8[v] is None:
            nrm8[v] = pl["nrm"].tile([P, KH, BN], FP8, tag=f"nrm{v}",
                                     name=f"nrm{v}_{r}")
        sq = [pl["sq"].tile([P, G], BF16, tag=f"sq{h}",
                            name=f"sq{v}{g}{h}_{r}") for h in range(KH)]
        for h in range(KH):
            nc.gpsimd.tensor_mul(sq[h][:], raw[v][h][:, gs], raw[v][h][:, gs])
        rb = pl["vec"].tile([P, G], F32, tag="rb", name=f"rb{v}{g}_{r}")
        for j in range(G // 512):
            js = slice(j * 512, (j + 1) * 512)
            ssb = pl["ss"].tile([P, 512], F32, tag="ss", name=f"ss{v}{g}{j}_{r}")
            for h in range(KH):
                nc.tensor.matmul(ssb[:], ones01[:], sq[h][:, js],
                                 start=(h == 0), stop=(h == KH - 1))
            nc.vector.reciprocal(rb[:, js], ssb[:])
        scl = pl["scl"].tile([P, G], BF16, tag="scl", name=f"scl{v}{g}_{r}")
        nc.scalar.activation(scl[:], rb[:], mybir.ActivationFunctionType.Sqrt)
        for h in range(KH):
            nc.gpsimd.tensor_mul(nrm8[v][:, h, gs], raw[v][h][:, gs], scl[:])


def _emit_main_g(nc, pl, r, g, nrm8, stats, diag01, diag00):
    ident, ones01, ones_col = pl["consts"]
    for half in range(2):       # 0: v0 rows, 1: v1 rows
        for m in range(4):      # 128-row blocks of this core's chunk
            hm = half * 4 + m
            ms = slice(m * P, (m + 1) * P)
            lhsT = nrm8[half][:, :, ms]
            for tg in range(2):  # 0: cross-view keys, 1: same-view
                keys = nrm8[1 - half] if tg == 0 else nrm8[half]
                for c in range(G // HT):  # half-tiles of this column group
                    coff = g * G + c * HT
                    pt = pl["pt"].tile([P, HT], F32, tag="pt",
                                       name=f"pt{g}{hm}{tg}{c}_{r}")
                    for j in range(HT // 512):
                        js = slice(j * 512, (j + 1) * 512)
                        nc.tensor.matmul(
                            pt[:, js], lhsT,
                            keys[:, :, coff + j * 512:coff + (j + 1) * 512],
                            perf_mode=mybir.MatmulPerfMode.DoubleRow)
                    if g == 0 and c == 0:
                        # tg==0: label logit (cross-view diag); tg==1:
                        # same-view diag (removed from row-sum later)
                        dst = diag01 if tg == 0 else diag00
                        dsc = pl["dsc"].tile([P, P], BF16, tag="dsc",
                                             name=f"dsc{hm}{tg}_{r}")
                        nc.vector.scalar_tensor_tensor(
                            dsc[:], pt[:, ms], 1.0, ident[:],
                            op0=mybir.AluOpType.mult,
                            op1=mybir.AluOpType.mult,
                            accum_out=dst[:, hm:hm + 1])
                    sidx = hm * 8 + tg * 4 + g * 2 + c
                    if ASSIGN[(g, half, m, tg, c)] == 'A':
                        esc = pl["esc"].tile([P, HT], BF16, tag="esc",
                                             name=f"esc{g}{hm}{tg}{c}_{r}")
                        nc.scalar.activation(
                            esc[:], pt[:, :],
                            mybir.ActivationFunctionType.Exp,
                            accum_out=stats[:, sidx:sidx + 1])
                    else:
                        it = pl["i16"].tile([P, HT], I16, tag="i16",
                                            name=f"it{g}{hm}{tg}{c}_{r}")
                        nc.vector.tensor_scalar(
                            it[:], pt[:, :], SCH_A, SCH_B,
                            op0=mybir.AluOpType.mult,
                            op1=mybir.AluOpType.add)
                        bfv = it[:].bitcast(BF16)
                        nc.vector.tensor_scalar(
                            bfv, bfv, 1.0, 0.0,
                            op0=mybir.AluOpType.mult,
                            op1=mybir.AluOpType.add,
                            accum_out=stats[:, sidx:sidx + 1])


def _emit_epilogue(nc, pl, out_dram, r, stats, diag01, diag00):
    ident, ones01, ones_col = pl["consts"]
    ediag = pl["sml"].tile([P, 8], F32, tag="ediag", name=f"ediag{r}")
    nc.scalar.activation(ediag[:], diag00[:], mybir.ActivationFunctionType.Exp)
    rows = pl["sml"].tile([P, 8], F32, tag="rows", name=f"rows{r}")
    nc.vector.tensor_reduce(
        rows[:], stats[:].rearrange("p (m t) -> p m t", t=8),
        axis=mybir.AxisListType.X, op=mybir.AluOpType.add)
    nc.vector.tensor_sub(rows[:], rows[:], ediag[:])
    lnr = pl["sml"].tile([P, 8], F32, tag="lnr", name=f"lnr{r}")
    lnsum = pl["sml"].tile([P, 1], F32, tag="lnsum", name=f"lnsum{r}")
    nc.scalar.activation(
        lnr[:], rows[:], mybir.ActivationFunctionType.Ln, accum_out=lnsum[:])
    dsum = pl["sml"].tile([P, 1], F32, tag="dsum", name=f"dsum{r}")
    nc.vector.tensor_reduce(
        dsum[:], diag01[:], axis=mybir.AxisListType.X, op=mybir.AluOpType.add)
    tot = pl["sml"].tile([P, 1], F32, tag="tot", name=f"tot{r}")
    nc.vector.tensor_sub(tot[:], lnsum[:], dsum[:])
    fp = pl["ss"].tile([P, 512], F32, tag="ss", name=f"fp{r}")
    nc.tensor.matmul(fp[0:1, 0:1], tot[:], ones_col[:])
    res = pl["sml"].tile([1, 1], F32, tag="res", name=f"res{r}")
    nc.vector.tensor_copy(res[:], fp[0:1, 0:1])
    nc.sync.dma_start(out=out_dram[:], in_=res[:])


def _emit_pass(nc, pl, vin, out_dram, r, do_setup=True, do_main=True,
               nrm_prev=None):
    """One full loss computation (rep r). Column-group outer: group-1
    normalize overlaps group-0 logits."""
    if do_setup:
        raw = [[None] * KH for _ in range(2)]
        nrm8 = [None, None]
        for g in range(NG):
            _emit_load_g(nc, pl, vin, r, g, raw)
    else:
        nrm8 = nrm_prev
    if do_main:
        stats = pl["sml"].tile([P, 64], F32, tag="stats", name=f"stats{r}")
        diag01 = pl["sml"].tile([P, 8], F32, tag="diag01", name=f"diag01{r}")
        diag00 = pl["sml"].tile([P, 8], F32, tag="diag00", name=f"diag00{r}")
    for g in range(NG):
        if do_setup:
            _emit_norm_g(nc, pl, r, g, raw, nrm8)
        if do_main:
            _emit_main_g(nc, pl, r, g, nrm8, stats, diag01, diag00)
    if do_main:
        _emit_epilogue(nc, pl, out_dram, r, stats, diag01, diag00)
    return nrm8


def _emit_setup(nc, pl, vin, r):
    return _emit_pass(nc, pl, vin, None, r, do_setup=True, do_main=False)


def _emit_main(nc, pl, out_dram, r, nrm8):
    _emit_pass(nc, pl, None, out_dram, r, do_setup=False, do_main=True,
               nrm_prev=nrm8)


def _build_nc(reps: int = 1, mode: str = "full"):
    """mode: 'full' reps everything; 'main' reps only the logits+exp phase
    (one shared setup); 'setup' reps only load+normalize; 'loop*' wraps the
    phase in a runtime-bounded hardware loop (niter input tensor)."""
    nc = bacc.Bacc()
    vin = [
        nc.dram_tensor("view0", [B, E, N], F32, kind="ExternalInput"),
        nc.dram_tensor("view1", [B, E, N], F32, kind="ExternalInput"),
    ]
    ident_in = nc.dram_tensor("ident", [P, P], F32, kind="ExternalInput")
    nit_in = None
    if mode.startswith("loop"):
        nit_in = nc.dram_tensor("niter", [1, 1], mybir.dt.int32,
                                kind="ExternalInput")
    out_dram = nc.dram_tensor("out", [1, 1], F32, kind="ExternalOutput")

    with ExitStack() as ctx:
        tc = ctx.enter_context(tile.TileContext(nc))
        pl = {
            name: ctx.enter_context(tc.tile_pool(name=name, bufs=bufs))
            for name, bufs in (("raw", 1), ("sq", 2), ("nrm", 2), ("vec", 2),
                               ("scl", 2), ("esc", 2), ("i16", 2), ("dsc", 2),
                               ("sml", 1))
        }
        pl["pt"] = ctx.enter_context(
            tc.tile_pool(name="pt", bufs=3, space="PSUM"))
        pl["ss"] = ctx.enter_context(
            tc.tile_pool(name="ss", bufs=2, space="PSUM"))

        ident = pl["sml"].tile([P, P], F32, tag="ident", name="ident")
        nc.sync.dma_start(out=ident[:], in_=ident_in[:])
        ones01 = pl["sml"].tile([P, P], BF16, tag="ones01", name="ones01")
        nc.vector.memset(ones01[:], 0.1)
        ones_col = pl["sml"].tile([P, 1], F32, tag="ones_col", name="ones_col")
        nc.vector.memset(ones_col[:], 1.0)
        pl["consts"] = (ident, ones01, ones_col)

        if mode.startswith("loop"):
            nt = pl["sml"].tile([1, 1], mybir.dt.int32, tag="nit", name="nit")
            nc.sync.dma_start(out=nt[:], in_=nit_in[:])
            n = nc.values_load(nt[:], min_val=1, max_val=1 << 20,
                               skip_runtime_bounds_check=True)
            sub = mode[5:] if len(mode) > 4 else "full"
            if sub == "main":
                nrm = _emit_setup(nc, pl, vin, 0)
                with tc.For_i(0, n):
                    _emit_main(nc, pl, out_dram, 0, nrm)
            elif sub == "setup":
                with tc.For_i(0, n):
                    _emit_setup(nc, pl, vin, 0)
                nc.sync.dma_start(out=out_dram[:],
                                  in_=pl["consts"][2][0:1, 0:1])
            elif sub == "nop":
                z = pl["sml"].tile([1, 1], F32, tag="z", name="z")
                with tc.For_i(0, n):
                    nc.vector.memset(z[:], 0.0)
                nc.sync.dma_start(out=out_dram[:], in_=z[:])
            else:
                with tc.For_i(0, n):
                    _emit_pass(nc, pl, vin, out_dram, 0)
        else:
            nrm = None
            for r in range(reps):
                nrm = _emit_pass(
                    nc, pl, vin, out_dram, r,
                    do_setup=(mode != "main" or r == 0),
                    do_main=(mode != "setup"),
                    nrm_prev=nrm)

    nc.compile()
    return nc


_NC_CACHE = None


def _run_spmd(view0: np.ndarray, view1: np.ndarray, nc=None, **spmd_kwargs):
    global _NC_CACHE
    if nc is None:
        if _NC_CACHE is None:
            _NC_CACHE = _build_nc()
        nc = _NC_CACHE

    ident = np.eye(P, dtype=np.float32)
    in_maps = []
    for c in range(NCORES):
        in_maps.append({
            "view0": np.ascontiguousarray(np.roll(view0, -c * (B // NCORES), axis=0)),
            "view1": np.ascontiguousarray(np.roll(view1, -c * (B // NCORES), axis=0)),
            "ident": ident,
        })
    res = run_bass_kernel_spmd(nc, in_maps, core_ids=list(range(NCORES)),
                               **spmd_kwargs)
    total = sum(float(r["out"][0, 0]) for r in res.results)
    return np.float32(total / (2 * BN)), res


def kernel(view0: np.ndarray, view1: np.ndarray) -> np.ndarray:
    loss, _ = _run_spmd(view0, view1)
    return loss
